# revision 7
# baseline (speedup 1.0000x reference)
"""MultiLinearUpsampling Trainium2 kernel.

Problem: out[b, t, :] = W[lidx[t]] @ pooled[b, segc[t], :]  (zero for invalid t)
where segc/lidx derive from sorted pooling_indices (ragged segments).

Strategy: output-dim split, region-major schedule, chunk-granular DMA
-------------------------------------------------------------------
Only sum_l N_l matvecs are unique per batch (N_l = #segments with
len > l).  Sorting segments by length (desc) makes each linear l's
column set a PREFIX of one flat (rank, batch) column axis: linear l
applies to flat columns [0, 8*N_l).

Each of the 8 cores owns a 128-row slice of D_out and computes ALL
columns for ALL 16 linears on that slice: perfectly balanced (exactly
sum_l 8*N_l = ~228k PE columns per core, no packing waste), and X is
one shared gather.  Per core per l: Y_l[m, c] = sum_d W[l, m_slice, d]
* X[d, c] for c < 8*N_l, as 8 contraction tiles x <=506-wide PSUM
windows (512 exactly throttles the PE clock), fp16 in / fp32
accumulate / fp16 out.

Two X column regions (A = first-linear prefix rounded to srt[3], B =
the rest), region-major: every linear's region-A windows run before
any region-B window.  X is staged per (region, k-chunk) as separate
DRAM tensors pre-swizzled on host to the exact SBUF layout
(per-partition contiguous), DMA'd in consumption order on the sync
ring, so the k-loop of the first window group starts as soon as chunk
0 + the first weight half-plane land (~3.4us after engine start)
instead of after the whole region.  W planes stream on the scalar
ring in process order.  Scratch warm-up matmuls bridge engine start
to first-data-ready so the PE clock (0.65GHz cold, 2.4GHz after
~3.4us of sustained activity) ramps exactly once.  Region B runs
largest-linear first so the final output drain is the smallest block;
Y drains ride the scalar ring mid-kernel and the (idle by then) sync
ring for the final block.

The host scatters Y columns to their t positions (including the
l = L-1 tail replication) and zero-fills invalid t.
"""

from contextlib import ExitStack

import numpy as np

import concourse.bass as bass  # noqa: F401  (bass types via bacc)
import concourse.mybir as mybir
import concourse.tile as tile
from concourse import bacc
from concourse.bass_utils import run_bass_kernel_spmd

F32 = mybir.dt.float32
F16 = mybir.dt.float16

B = 8          # batch
N = 512        # segments
D = 1024       # D_in == D_out
L = 16         # linears
NCORES = 8
KC = 8         # contraction chunks of 128
MSLICE = 128   # out-dim rows per core
WMAX = 512     # PSUM tile width (one bank of fp32); matmul windows are
               # capped at 506 -- 512-wide matmuls throttle the PE clock
NWARM_128 = 20  # scratch warm-up matmuls (128-wide, ~107ns each cold)
NWARM_256 = 2   # plus a couple of 256-wide for cushion


def _bounds(F_ls):
    """X region boundaries [0, bA, F]: region A is a prefix every linear
    reuses (srt[3] keeps fragmentation low while letting the smallest
    linears complete entirely in region A)."""
    srt = sorted(F_ls)
    F = srt[-1]
    cand = {F}
    if len(srt) > 4:
        cand.add(srt[3])
    return [0] + sorted(c for c in cand if c > 0)


def _windows(F_l, bounds):
    """Split [0, F_l) at region bounds, even-split into <=506 windows
    (512 exactly throttles the PE clock).  Returns (region, start, len)
    with start relative to region start."""
    out = []
    for ri in range(len(bounds) - 1):
        lo, hi = bounds[ri], min(bounds[ri + 1], F_l)
        if hi <= lo:
            break
        width = hi - lo
        nw = -(-width // 506)
        base, rem = divmod(width, nw)
        off = 0
        for j in range(nw):
            ln = base + (1 if j < rem else 0)
            out.append((ri, off, ln))
            off += ln
    return out


def _build_program(F_ls):
    """F_ls: per-linear flat-column prefix lengths (ascending process
    order, zeros removed)."""
    nc = bacc.Bacc("TRN2", target_bir_lowering=False, debug=False)
    bounds = _bounds(F_ls)
    nreg = len(bounds) - 1
    widths = [bounds[i + 1] - bounds[i] for i in range(nreg)]
    NL = len(F_ls)
    total_cols = sum(F_ls)

    # per-(region, chunk) X tensors, host-pre-swizzled to (128, w):
    # partition-contiguous so each DMA is a clean 16-engine streaming copy
    xs = {
        (r, k): nc.dram_tensor(f"x{r}_{k}", (128, widths[r]), F16,
                               kind="ExternalInput")
        for r in range(nreg)
        for k in range(KC)
    }
    wh = nc.dram_tensor("w", (NL, 128, KC * 128), F16, kind="ExternalInput")
    y = nc.dram_tensor("y", (128, total_cols), F16, kind="ExternalOutput")

    with tile.TileContext(nc) as tc, ExitStack() as ctx:
        xpool = ctx.enter_context(tc.tile_pool(name="x", bufs=1))
        wpool = ctx.enter_context(tc.tile_pool(name="w", bufs=1))
        ypool = ctx.enter_context(tc.tile_pool(name="y", bufs=1))
        ppool = ctx.enter_context(tc.tile_pool(name="ps", bufs=8, space="PSUM"))

        # scratch operands for the PE warm-up (memset first so the DVE
        # dispatches them at the earliest scheduler tick)
        sx = xpool.tile([128, 256], F16, tag="scratch_x", name="sx")
        sw = xpool.tile([128, 128], F16, tag="scratch_w", name="sw")
        nc.vector.memset(sx[:], 0.0)
        nc.vector.memset(sw[:], 0.0)

        # ---- DMA program: issued first so the rings start streaming
        # immediately.  The startup-critical bytes all ride the sync
        # ring as ONE strictly-ordered stream matched to consumption
        # order (cross-ring packet round-robin is not byte-fair, so a
        # critical W plane on the other ring can starve behind bulk X):
        # region-A chunks interleaved with the first 5 W planes, then
        # region-B chunks.  W planes 5+ trickle on the scalar ring.
        xt = {}
        for r in range(nreg):
            for k in range(KC):
                xt[(r, k)] = xpool.tile(
                    [128, widths[r]], F16, tag=f"x{r}_{k}", name=f"x{r}_{k}"
                )

        wt = {}   # li -> list of (tile, k_lo, k_hi)
        w0a = wpool.tile([128, 4 * 128], F16, tag="w0a", name="w0a")
        w0b = wpool.tile([128, 4 * 128], F16, tag="w0b", name="w0b")
        wt[0] = [(w0a, 0, 4), (w0b, 4, 8)]
        for li in range(1, NL):
            t = wpool.tile([128, KC * 128], F16, tag=f"w{li}", name=f"w{li}")
            wt[li] = [(t, 0, KC)]

        def dma_w(li):
            if li == 0:
                nc.sync.dma_start(w0a[:], wh.ap()[0, :, : 4 * 128])
                nc.sync.dma_start(w0b[:], wh.ap()[0, :, 4 * 128 :])
            else:
                nc.sync.dma_start(wt[li][0][0][:], wh.ap()[li])

        NW_SYNC = min(5, NL)   # planes interleaved into the sync stream
        sync_order = [("x", 0, 0), ("w", 0), ("w", 1), ("x", 0, 1),
                      ("x", 0, 2), ("w", 2), ("x", 0, 3), ("w", 3),
                      ("x", 0, 4), ("w", 4), ("x", 0, 5), ("x", 0, 6),
                      ("x", 0, 7)]
        for item in sync_order:
            if item[0] == "x":
                _, r, k = item
                nc.sync.dma_start(xt[(r, k)][:], xs[(r, k)].ap())
            elif item[1] < NL:
                dma_w(item[1])
        for r in range(1, nreg):
            for k in range(KC):
                nc.sync.dma_start(xt[(r, k)][:], xs[(r, k)].ap())
        for li in range(NW_SYNC, NL):
            nc.scalar.dma_start(wt[li][0][0][:], wh.ap()[li])

        def wchunk(li, k):
            for t, klo, khi in wt[li]:
                if klo <= k < khi:
                    return t[:, (k - klo) * 128 : (k - klo + 1) * 128]
            raise AssertionError

        # ---- PE clock warm-up: any PE idle gap >~3.4us drops the clock
        # to 0.65GHz with a ~3.4us re-ramp, so keep the array busy on
        # scratch matmuls from kernel start until the first real window's
        # data has landed -- the ramp credit then carries into real work.
        spt = ppool.tile([128, WMAX], F32, tag="ps", name="spt")
        for _ in range(NWARM_128):
            nc.tensor.matmul(spt[:, :128], sw[:], sx[:, :128], start=True, stop=True)
        for _ in range(NWARM_256):
            nc.tensor.matmul(spt[:, :256], sw[:], sx[:], start=True, stop=True)

        # column offsets of each linear's Y block in the output tensor
        offs = []
        off = 0
        for li in range(NL):
            offs.append(off)
            off += F_ls[li]

        # ---- Region-major schedule: all linears' region-A windows run
        # before any region-B window.  Region-A windows are grouped in
        # fours ACROSS consecutive linears so each k-step burns ~4x500
        # PE columns per X chunk -- the PE stays ahead of the incoming
        # chunk stream at startup instead of chasing it.  Region B runs
        # largest-first so the final Y drain is the smallest block.
        yts = {}
        ydrained = {li: 0 for li in range(NL)}
        winsl = {li: _windows(F_ls[li], bounds) for li in range(NL)}

        for r in range(nreg):
            order = [li for li in range(NL) if F_ls[li] > bounds[r]]
            if r == nreg - 1 and len(order) > 1:
                order = order[::-1]
            # flat (li, ws, wl) sequence for this region, in linear order
            seq = []
            for li in order:
                if li not in yts:
                    yts[li] = ypool.tile(
                        [128, F_ls[li]], F16, tag=f"yt{li}", name=f"yt{li}"
                    )
                for (ri, ws, wl) in winsl[li]:
                    if ri == r:
                        seq.append((li, ws, wl))
            remaining = {li: sum(1 for s in seq if s[0] == li) for li in order}
            for g0 in range(0, len(seq), 4):
                grp = seq[g0 : g0 + 4]
                pts = [
                    ppool.tile(
                        [128, WMAX], F32, tag="ps", name=f"ps{r}_{g0}_{j}"
                    )
                    for j in range(len(grp))
                ]
                for k in range(KC):
                    for (li, ws, wl), pt in zip(grp, pts):
                        nc.tensor.matmul(
                            pt[:, :wl],
                            wchunk(li, k),
                            xt[(r, k)][:, ws : ws + wl],
                            start=(k == 0),
                            stop=(k == KC - 1),
                        )
                for (li, ws, wl), pt in zip(grp, pts):
                    woff = bounds[r] + ws
                    nc.vector.tensor_copy(
                        yts[li][:, woff : woff + wl], pt[:, :wl]
                    )
                # drain any linear whose region-r windows all completed;
                # finish on its last region.  Mid-kernel drains ride the
                # scalar ring; the final block of the program goes on the
                # (idle by then) sync ring.
                for (li, ws, wl) in grp:
                    remaining[li] -= 1
                for li in {s[0] for s in grp}:
                    if remaining[li] == 0:
                        done = min(bounds[r + 1], F_ls[li])
                        last = done >= F_ls[li]
                        dr = ydrained[li]
                        if last or done - dr >= 688:
                            is_final = r == nreg - 1 and li == order[-1]
                            eng = nc.sync if is_final else nc.scalar
                            eng.dma_start(
                                y.ap()[:, offs[li] + dr : offs[li] + done],
                                yts[li][:, dr:done],
                            )
                            ydrained[li] = done
                        remaining[li] = -1

    nc.compile()
    return nc


# ---------------------------------------------------------------------------
# host wrapper
# ---------------------------------------------------------------------------

def _segment_structure(idx, T):
    t = np.arange(T)
    seg = np.searchsorted(idx, t, side="left")
    valid = seg < N
    segc = np.clip(seg, 0, N - 1)
    start = np.where(segc > 0, idx[np.maximum(segc - 1, 0)] + 1, 0)
    lidx = np.minimum(t - start, L - 1).astype(np.int64)
    lens = np.bincount(segc[valid], minlength=N)
    return t, seg, valid, segc, lidx, lens


def _install_ntff_hook():
    """Profiling-only: register the axon NTFF profile hook (dev use)."""
    import sys
    import types

    try:
        import antenv

        if "antenv.axon_hooks" not in sys.modules:
            mod = types.ModuleType("antenv.axon_hooks")
            holder = [None]
            mod.set_axon_ntff_profile_hook = lambda h: holder.__setitem__(0, h)
            mod.get_axon_ntff_profile_hook = lambda: holder[0]
            sys.modules["antenv.axon_hooks"] = mod
            antenv.axon_hooks = mod
            from trn_agent_boot.trn_boot import _ntff_profile_via_ctypes

            mod.set_axon_ntff_profile_hook(
                _ntff_profile_via_ctypes("/opt/axon/libaxon_pjrt.so")
            )
    except Exception as e:
        print(f"NTFF hook install failed: {e}")


def kernel(pooled_vectors, W, pooling_indices, target_length, _trace=False):
    pooled = np.asarray(pooled_vectors, dtype=np.float32)
    Wf = np.asarray(W, dtype=np.float32)
    idx = np.asarray(pooling_indices).astype(np.int64)
    T = int(np.asarray(target_length))

    t, seg, valid, segc, lidx, lens = _segment_structure(idx, T)

    order = np.argsort(-lens, kind="stable")      # segments by len desc
    rank_of_seg = np.empty(N, dtype=np.int64)
    rank_of_seg[order] = np.arange(N)
    N_l = (lens[None, :] > np.arange(L)[:, None]).sum(axis=1)  # (L,)

    # process order: ascending prefix length (early compute only needs
    # early X chunks); zero-size linears skipped
    proc = [l for l in np.argsort(N_l, kind="stable") if N_l[l] > 0]
    F_ls = [8 * int(N_l[l]) for l in proc]
    F = max(F_ls) if F_ls else 8

    nc = _build_program(F_ls)

    # flat column axis: (rank-major, batch-minor), ranks with len>0 only
    n0 = F // 8
    Xh = (
        pooled.transpose(2, 1, 0)[:, order[:n0], :]
        .reshape(D, F)
        .astype(np.float16)
    )  # (D, F), col = r*8 + b
    bounds = _bounds(F_ls)
    nreg = len(bounds) - 1
    # per-(region, chunk) blocks, each (128, w) partition-contiguous
    xblocks = {}
    for r in range(nreg):
        blk = Xh[:, bounds[r] : bounds[r + 1]]          # (1024, w)
        for k in range(KC):
            xblocks[(r, k)] = np.ascontiguousarray(blk[k * 128 : (k + 1) * 128])

    in_maps = []
    for c in range(NCORES):
        # W[l, m_slice, d] -> (l, kp, kc*128+m) with d = kc*128 + kp
        wc = (
            Wf[np.array(proc), c * 128 : (c + 1) * 128, :]
            .transpose(0, 2, 1)
            .reshape(len(proc), KC, 128, 128)
            .transpose(0, 2, 1, 3)
            .reshape(len(proc), 128, KC * 128)
            .astype(np.float16)
        )
        im = {"w": np.ascontiguousarray(wc)}
        for (r, k), xb in xblocks.items():
            im[f"x{r}_{k}"] = xb
        in_maps.append(im)

    kwargs = {}
    if _trace:
        _install_ntff_hook()
        kwargs = dict(trace=True)
    res = run_bass_kernel_spmd(nc, in_maps, core_ids=list(range(NCORES)), **kwargs)
    results = res.results

    # assemble (D, total_cols) then scatter to (B, T, D)
    Yall = np.concatenate(
        [np.asarray(results[c]["y"]) for c in range(NCORES)], axis=0
    )  # (1024, total_cols) f16
    col_off = np.zeros(L, dtype=np.int64)
    off = 0
    for li, l in enumerate(proc):
        col_off[l] = off
        off += F_ls[li]

    Dout = Wf.shape[1]
    out = np.zeros((B, T, Dout), dtype=np.float32)
    tv = t[valid]
    ci = col_off[lidx[tv]] + rank_of_seg[segc[tv]] * 8  # (Tv,)
    cib = ci[:, None] + np.arange(B)[None, :]           # (Tv, B)
    out[:, tv, :] = Yall[:, cib].transpose(2, 1, 0).astype(np.float32)

    if _trace:
        kernel._last_exec_time_ns = res.exec_time_ns
        kernel._last_results = res
    return out


# revision 8
# speedup vs baseline: 1.0246x; 1.0246x over previous
"""MultiLinearUpsampling Trainium2 kernel.

Problem: out[b, t, :] = W[lidx[t]] @ pooled[b, segc[t], :]  (zero for invalid t)
where segc/lidx derive from sorted pooling_indices (ragged segments).

Strategy: output-dim split, region-major schedule, chunk-granular DMA
-------------------------------------------------------------------
Only sum_l N_l matvecs are unique per batch (N_l = #segments with
len > l).  Sorting segments by length (desc) makes each linear l's
column set a PREFIX of one flat (rank, batch) column axis: linear l
applies to flat columns [0, 8*N_l).

Each of the 8 cores owns a 128-row slice of D_out and computes ALL
columns for ALL 16 linears on that slice: perfectly balanced (exactly
sum_l 8*N_l = ~228k PE columns per core, no packing waste), and X is
one shared gather.  Per core per l: Y_l[m, c] = sum_d W[l, m_slice, d]
* X[d, c] for c < 8*N_l, as 8 contraction tiles x <=506-wide PSUM
windows (512 exactly throttles the PE clock), fp16 in / fp32
accumulate / fp16 out.

Two X column regions (A = first-linear prefix rounded to srt[3], B =
the rest), region-major: every linear's region-A windows run before
any region-B window.  X is staged per (region, k-chunk) as separate
DRAM tensors pre-swizzled on host to the exact SBUF layout
(per-partition contiguous), DMA'd in consumption order on the sync
ring, so the k-loop of the first window group starts as soon as chunk
0 + the first weight half-plane land (~3.4us after engine start)
instead of after the whole region.  W planes stream on the scalar
ring in process order.  Scratch warm-up matmuls bridge engine start
to first-data-ready so the PE clock (0.65GHz cold, 2.4GHz after
~3.4us of sustained activity) ramps exactly once.  Region B runs
largest-linear first so the final output drain is the smallest block;
Y drains ride the scalar ring mid-kernel and the (idle by then) sync
ring for the final block.

The host scatters Y columns to their t positions (including the
l = L-1 tail replication) and zero-fills invalid t.
"""

from contextlib import ExitStack

import numpy as np

import concourse.bass as bass  # noqa: F401  (bass types via bacc)
import concourse.mybir as mybir
import concourse.tile as tile
from concourse import bacc
from concourse.bass_utils import run_bass_kernel_spmd

F32 = mybir.dt.float32
F16 = mybir.dt.float16

B = 8          # batch
N = 512        # segments
D = 1024       # D_in == D_out
L = 16         # linears
NCORES = 8
KC = 8         # contraction chunks of 128
MSLICE = 128   # out-dim rows per core
WMAX = 512     # PSUM tile width (one bank of fp32); matmul windows are
               # capped at 506 -- 512-wide matmuls throttle the PE clock
NWARM_128 = 20  # scratch warm-up matmuls (128-wide, ~107ns each cold)
NWARM_256 = 2   # plus a couple of 256-wide for cushion


def _bounds(F_ls):
    """X region boundaries [0, bA, F]: region A is a prefix every linear
    reuses (srt[3] keeps fragmentation low while letting the smallest
    linears complete entirely in region A)."""
    srt = sorted(F_ls)
    F = srt[-1]
    cand = {F}
    if len(srt) > 4:
        cand.add(srt[3])
    return [0] + sorted(c for c in cand if c > 0)


def _windows(F_l, bounds):
    """Split [0, F_l) at region bounds, even-split into <=506 windows
    (512 exactly throttles the PE clock).  Returns (region, start, len)
    with start relative to region start."""
    out = []
    for ri in range(len(bounds) - 1):
        lo, hi = bounds[ri], min(bounds[ri + 1], F_l)
        if hi <= lo:
            break
        width = hi - lo
        nw = -(-width // 506)
        base, rem = divmod(width, nw)
        off = 0
        for j in range(nw):
            ln = base + (1 if j < rem else 0)
            out.append((ri, off, ln))
            off += ln
    return out


def _build_program(F_ls):
    """F_ls: per-linear flat-column prefix lengths (ascending process
    order, zeros removed)."""
    nc = bacc.Bacc("TRN2", target_bir_lowering=False, debug=False)
    bounds = _bounds(F_ls)
    nreg = len(bounds) - 1
    widths = [bounds[i + 1] - bounds[i] for i in range(nreg)]
    NL = len(F_ls)
    total_cols = sum(F_ls)

    # per-(region, chunk) X tensors, host-pre-swizzled to (128, w):
    # partition-contiguous so each DMA is a clean 16-engine streaming copy
    xs = {
        (r, k): nc.dram_tensor(f"x{r}_{k}", (128, widths[r]), F16,
                               kind="ExternalInput")
        for r in range(nreg)
        for k in range(KC)
    }
    wh = nc.dram_tensor("w", (NL, 128, KC * 128), F16, kind="ExternalInput")
    y = nc.dram_tensor("y", (128, total_cols), F16, kind="ExternalOutput")

    with tile.TileContext(nc) as tc, ExitStack() as ctx:
        xpool = ctx.enter_context(tc.tile_pool(name="x", bufs=1))
        wpool = ctx.enter_context(tc.tile_pool(name="w", bufs=1))
        ypool = ctx.enter_context(tc.tile_pool(name="y", bufs=1))
        ppool = ctx.enter_context(tc.tile_pool(name="ps", bufs=8, space="PSUM"))

        # scratch operands for the PE warm-up (memset first so the DVE
        # dispatches them at the earliest scheduler tick)
        sx = xpool.tile([128, 256], F16, tag="scratch_x", name="sx")
        sw = xpool.tile([128, 128], F16, tag="scratch_w", name="sw")
        nc.vector.memset(sx[:], 0.0)
        nc.vector.memset(sw[:], 0.0)

        # ---- DMA program: issued first so the rings start streaming
        # immediately.  Each dma_start costs ~0.65us of serialized
        # descriptor generation on its ring, and the two HWDGE rings
        # round-robin the 16 SDMA engines at packet granularity, so
        # ring position IS priority: startup-critical bytes go first on
        # their ring and everything else queues BEHIND them on the same
        # rings (a separate "background" ring would steal bandwidth).
        #   sync:   region-A chunks k0..k7, w2, w3, region-B chunks
        #   scalar: w0 (halved so the first k-steps start early), w1,
        #           w4..w15, then the Y drains as the schedule emits them
        xt = {}
        for r in range(nreg):
            for k in range(KC):
                xt[(r, k)] = xpool.tile(
                    [128, widths[r]], F16, tag=f"x{r}_{k}", name=f"x{r}_{k}"
                )

        wt = {}   # li -> list of (tile, k_lo, k_hi)
        w0a = wpool.tile([128, 4 * 128], F16, tag="w0a", name="w0a")
        w0b = wpool.tile([128, 4 * 128], F16, tag="w0b", name="w0b")
        wt[0] = [(w0a, 0, 4), (w0b, 4, 8)]
        for li in range(1, NL):
            t = wpool.tile([128, KC * 128], F16, tag=f"w{li}", name=f"w{li}")
            wt[li] = [(t, 0, KC)]

        SYNC_W = [li for li in (2, 3) if li < NL]
        for k in range(KC):
            nc.sync.dma_start(xt[(0, k)][:], xs[(0, k)].ap())
        for li in SYNC_W:
            nc.sync.dma_start(wt[li][0][0][:], wh.ap()[li])
        for r in range(1, nreg):
            for k in range(KC):
                nc.sync.dma_start(xt[(r, k)][:], xs[(r, k)].ap())

        nc.scalar.dma_start(w0a[:], wh.ap()[0, :, : 4 * 128])
        nc.scalar.dma_start(w0b[:], wh.ap()[0, :, 4 * 128 :])
        for li in range(1, NL):
            if li not in SYNC_W:
                nc.scalar.dma_start(wt[li][0][0][:], wh.ap()[li])

        def wchunk(li, k):
            for t, klo, khi in wt[li]:
                if klo <= k < khi:
                    return t[:, (k - klo) * 128 : (k - klo + 1) * 128]
            raise AssertionError

        # ---- PE clock warm-up: any PE idle gap >~3.4us drops the clock
        # to 0.65GHz with a ~3.4us re-ramp, so keep the array busy on
        # scratch matmuls from kernel start until the first real window's
        # data has landed -- the ramp credit then carries into real work.
        spt = ppool.tile([128, WMAX], F32, tag="ps", name="spt")
        for _ in range(NWARM_128):
            nc.tensor.matmul(spt[:, :128], sw[:], sx[:, :128], start=True, stop=True)
        for _ in range(NWARM_256):
            nc.tensor.matmul(spt[:, :256], sw[:], sx[:], start=True, stop=True)

        # column offsets of each linear's Y block in the output tensor
        offs = []
        off = 0
        for li in range(NL):
            offs.append(off)
            off += F_ls[li]

        # ---- Region-major schedule: all linears' region-A windows run
        # before any region-B window.  Region-A windows are grouped in
        # fours ACROSS consecutive linears so each k-step burns ~4x500
        # PE columns per X chunk -- the PE stays ahead of the incoming
        # chunk stream at startup instead of chasing it.  Region B runs
        # largest-first so the final Y drain is the smallest block.
        yts = {}
        ydrained = {li: 0 for li in range(NL)}
        winsl = {li: _windows(F_ls[li], bounds) for li in range(NL)}

        for r in range(nreg):
            order = [li for li in range(NL) if F_ls[li] > bounds[r]]
            if r == nreg - 1 and len(order) > 1:
                order = order[::-1]
            # flat (li, ws, wl) sequence for this region, in linear order
            seq = []
            for li in order:
                if li not in yts:
                    yts[li] = ypool.tile(
                        [128, F_ls[li]], F16, tag=f"yt{li}", name=f"yt{li}"
                    )
                for (ri, ws, wl) in winsl[li]:
                    if ri == r:
                        seq.append((li, ws, wl))
            remaining = {li: sum(1 for s in seq if s[0] == li) for li in order}
            for g0 in range(0, len(seq), 4):
                grp = seq[g0 : g0 + 4]
                pts = [
                    ppool.tile(
                        [128, WMAX], F32, tag="ps", name=f"ps{r}_{g0}_{j}"
                    )
                    for j in range(len(grp))
                ]
                for k in range(KC):
                    for (li, ws, wl), pt in zip(grp, pts):
                        nc.tensor.matmul(
                            pt[:, :wl],
                            wchunk(li, k),
                            xt[(r, k)][:, ws : ws + wl],
                            start=(k == 0),
                            stop=(k == KC - 1),
                        )
                for (li, ws, wl), pt in zip(grp, pts):
                    woff = bounds[r] + ws
                    nc.vector.tensor_copy(
                        yts[li][:, woff : woff + wl], pt[:, :wl]
                    )
                # drain any linear whose region-r windows all completed;
                # finish on its last region.  Mid-kernel drains ride the
                # scalar ring; the final block of the program goes on the
                # (idle by then) sync ring.
                for (li, ws, wl) in grp:
                    remaining[li] -= 1
                for li in {s[0] for s in grp}:
                    if remaining[li] == 0:
                        done = min(bounds[r + 1], F_ls[li])
                        last = done >= F_ls[li]
                        dr = ydrained[li]
                        if last or done - dr >= 688:
                            is_final = r == nreg - 1 and li == order[-1]
                            eng = nc.sync if is_final else nc.scalar
                            eng.dma_start(
                                y.ap()[:, offs[li] + dr : offs[li] + done],
                                yts[li][:, dr:done],
                            )
                            ydrained[li] = done
                        remaining[li] = -1

    nc.compile()
    return nc


# ---------------------------------------------------------------------------
# host wrapper
# ---------------------------------------------------------------------------

def _segment_structure(idx, T):
    t = np.arange(T)
    seg = np.searchsorted(idx, t, side="left")
    valid = seg < N
    segc = np.clip(seg, 0, N - 1)
    start = np.where(segc > 0, idx[np.maximum(segc - 1, 0)] + 1, 0)
    lidx = np.minimum(t - start, L - 1).astype(np.int64)
    lens = np.bincount(segc[valid], minlength=N)
    return t, seg, valid, segc, lidx, lens


def _install_ntff_hook():
    """Profiling-only: register the axon NTFF profile hook (dev use)."""
    import sys
    import types

    try:
        import antenv

        if "antenv.axon_hooks" not in sys.modules:
            mod = types.ModuleType("antenv.axon_hooks")
            holder = [None]
            mod.set_axon_ntff_profile_hook = lambda h: holder.__setitem__(0, h)
            mod.get_axon_ntff_profile_hook = lambda: holder[0]
            sys.modules["antenv.axon_hooks"] = mod
            antenv.axon_hooks = mod
            from trn_agent_boot.trn_boot import _ntff_profile_via_ctypes

            mod.set_axon_ntff_profile_hook(
                _ntff_profile_via_ctypes("/opt/axon/libaxon_pjrt.so")
            )
    except Exception as e:
        print(f"NTFF hook install failed: {e}")


def kernel(pooled_vectors, W, pooling_indices, target_length, _trace=False):
    pooled = np.asarray(pooled_vectors, dtype=np.float32)
    Wf = np.asarray(W, dtype=np.float32)
    idx = np.asarray(pooling_indices).astype(np.int64)
    T = int(np.asarray(target_length))

    t, seg, valid, segc, lidx, lens = _segment_structure(idx, T)

    order = np.argsort(-lens, kind="stable")      # segments by len desc
    rank_of_seg = np.empty(N, dtype=np.int64)
    rank_of_seg[order] = np.arange(N)
    N_l = (lens[None, :] > np.arange(L)[:, None]).sum(axis=1)  # (L,)

    # process order: ascending prefix length (early compute only needs
    # early X chunks); zero-size linears skipped
    proc = [l for l in np.argsort(N_l, kind="stable") if N_l[l] > 0]
    F_ls = [8 * int(N_l[l]) for l in proc]
    F = max(F_ls) if F_ls else 8

    nc = _build_program(F_ls)

    # flat column axis: (rank-major, batch-minor), ranks with len>0 only
    n0 = F // 8
    Xh = (
        pooled.transpose(2, 1, 0)[:, order[:n0], :]
        .reshape(D, F)
        .astype(np.float16)
    )  # (D, F), col = r*8 + b
    bounds = _bounds(F_ls)
    nreg = len(bounds) - 1
    # per-(region, chunk) blocks, each (128, w) partition-contiguous
    xblocks = {}
    for r in range(nreg):
        blk = Xh[:, bounds[r] : bounds[r + 1]]          # (1024, w)
        for k in range(KC):
            xblocks[(r, k)] = np.ascontiguousarray(blk[k * 128 : (k + 1) * 128])

    in_maps = []
    for c in range(NCORES):
        # W[l, m_slice, d] -> (l, kp, kc*128+m) with d = kc*128 + kp
        wc = (
            Wf[np.array(proc), c * 128 : (c + 1) * 128, :]
            .transpose(0, 2, 1)
            .reshape(len(proc), KC, 128, 128)
            .transpose(0, 2, 1, 3)
            .reshape(len(proc), 128, KC * 128)
            .astype(np.float16)
        )
        im = {"w": np.ascontiguousarray(wc)}
        for (r, k), xb in xblocks.items():
            im[f"x{r}_{k}"] = xb
        in_maps.append(im)

    kwargs = {}
    if _trace:
        _install_ntff_hook()
        kwargs = dict(trace=True)
    res = run_bass_kernel_spmd(nc, in_maps, core_ids=list(range(NCORES)), **kwargs)
    results = res.results

    # assemble (D, total_cols) then scatter to (B, T, D)
    Yall = np.concatenate(
        [np.asarray(results[c]["y"]) for c in range(NCORES)], axis=0
    )  # (1024, total_cols) f16
    col_off = np.zeros(L, dtype=np.int64)
    off = 0
    for li, l in enumerate(proc):
        col_off[l] = off
        off += F_ls[li]

    Dout = Wf.shape[1]
    out = np.zeros((B, T, Dout), dtype=np.float32)
    tv = t[valid]
    ci = col_off[lidx[tv]] + rank_of_seg[segc[tv]] * 8  # (Tv,)
    cib = ci[:, None] + np.arange(B)[None, :]           # (Tv, B)
    out[:, tv, :] = Yall[:, cib].transpose(2, 1, 0).astype(np.float32)

    if _trace:
        kernel._last_exec_time_ns = res.exec_time_ns
        kernel._last_results = res
    return out


# revision 14
# speedup vs baseline: 1.0625x; 1.0369x over previous
"""MultiLinearUpsampling Trainium2 kernel.

Problem: out[b, t, :] = W[lidx[t]] @ pooled[b, segc[t], :]  (zero for invalid t)
where segc/lidx derive from sorted pooling_indices (ragged segments).

Strategy: output-dim split, region-major schedule, chunk-granular DMA
-------------------------------------------------------------------
Only sum_l N_l matvecs are unique per batch (N_l = #segments with
len > l).  Sorting segments by length (desc) makes each linear l's
column set a PREFIX of one flat (rank, batch) column axis: linear l
applies to flat columns [0, 8*N_l).

Each of the 8 cores owns a 128-row slice of D_out and computes ALL
columns for ALL 16 linears on that slice: perfectly balanced (exactly
sum_l 8*N_l = ~228k PE columns per core, no packing waste), and X is
one shared gather.  Per core per l: Y_l[m, c] = sum_d W[l, m_slice, d]
* X[d, c] for c < 8*N_l, as 8 contraction tiles x <=506-wide PSUM
windows (512 exactly throttles the PE clock), fp16 in / fp32
accumulate / fp16 out.

Two X column regions (A = first-linear prefix rounded to srt[3], B =
the rest), region-major: every linear's region-A windows run before
any region-B window.  X is staged per (region, k-chunk) as separate
DRAM tensors pre-swizzled on host to the exact SBUF layout
(per-partition contiguous), DMA'd in consumption order on the sync
ring, so the k-loop of the first window group starts as soon as chunk
0 + the first weight half-plane land (~3.4us after engine start)
instead of after the whole region.  W planes stream on the scalar
ring in process order.  Scratch warm-up matmuls bridge engine start
to first-data-ready so the PE clock (0.65GHz cold, 2.4GHz after
~3.4us of sustained activity) ramps exactly once.  Region B runs
largest-linear first so the final output drain is the smallest block;
Y drains ride the scalar ring mid-kernel and the (idle by then) sync
ring for the final block.

The host scatters Y columns to their t positions (including the
l = L-1 tail replication) and zero-fills invalid t.
"""

from contextlib import ExitStack

import numpy as np

import concourse.bass as bass  # noqa: F401  (bass types via bacc)
import concourse.mybir as mybir
import concourse.tile as tile
from concourse import bacc
from concourse.bass_utils import run_bass_kernel_spmd

F32 = mybir.dt.float32
F16 = mybir.dt.float16

B = 8          # batch
N = 512        # segments
D = 1024       # D_in == D_out
L = 16         # linears
NCORES = 8
KC = 8         # contraction chunks of 128
MSLICE = 128   # out-dim rows per core
WMAX = 512     # PSUM tile width (one bank of fp32); matmul windows are
               # capped at 506 -- 512-wide matmuls throttle the PE clock
NWARM_128 = 14  # scratch warm-up matmuls (128-wide, ~107ns each cold)
NWARM_256 = 1   # plus a 256-wide for cushion


def _bounds(F_ls):
    """X region boundaries [0, bA, F]: region A is a prefix every linear
    reuses (srt[3] keeps fragmentation low while letting the smallest
    linears complete entirely in region A)."""
    srt = sorted(F_ls)
    F = srt[-1]
    cand = {F}
    if len(srt) > 4:
        cand.add(srt[3])
    return [0] + sorted(c for c in cand if c > 0)


def _windows(F_l, bounds):
    """Split [0, F_l) at region bounds, even-split into <=506 windows
    (512 exactly throttles the PE clock).  Returns (region, start, len)
    with start relative to region start."""
    out = []
    for ri in range(len(bounds) - 1):
        lo, hi = bounds[ri], min(bounds[ri + 1], F_l)
        if hi <= lo:
            break
        width = hi - lo
        nw = -(-width // 506)
        base, rem = divmod(width, nw)
        off = 0
        for j in range(nw):
            ln = base + (1 if j < rem else 0)
            out.append((ri, off, ln))
            off += ln
    return out


def _build_program(F_ls):
    """F_ls: per-linear flat-column prefix lengths (ascending process
    order, zeros removed)."""
    nc = bacc.Bacc("TRN2", target_bir_lowering=False, debug=False)
    bounds = _bounds(F_ls)
    nreg = len(bounds) - 1
    widths = [bounds[i + 1] - bounds[i] for i in range(nreg)]
    NL = len(F_ls)
    total_cols = sum(F_ls)

    # per-(region, chunk-pair) X tensors, host-pre-swizzled to
    # (128, 2w): partition-contiguous so each DMA is a clean 16-engine
    # streaming copy
    xs = {
        (r, p): nc.dram_tensor(f"x{r}_{p}", (128, 2 * widths[r]), F16,
                               kind="ExternalInput")
        for r in range(nreg)
        for p in range(KC // 2)
    }
    wh = nc.dram_tensor("w", (NL, 128, KC * 128), F16, kind="ExternalInput")
    y = nc.dram_tensor("y", (128, total_cols), F16, kind="ExternalOutput")

    with tile.TileContext(nc) as tc, ExitStack() as ctx:
        xpool = ctx.enter_context(tc.tile_pool(name="x", bufs=1))
        wpool = ctx.enter_context(tc.tile_pool(name="w", bufs=1))
        ypool = ctx.enter_context(tc.tile_pool(name="y", bufs=1))
        ppool = ctx.enter_context(tc.tile_pool(name="ps", bufs=8, space="PSUM"))

        # scratch operands for the PE warm-up (memset on the otherwise
        # idle GpSimd so the DVE/PE path isn't gated on it)
        sx = xpool.tile([128, 256], F16, tag="scratch_x", name="sx")
        sw = xpool.tile([128, 128], F16, tag="scratch_w", name="sw")
        nc.gpsimd.memset(sx[:], 0.0)
        nc.gpsimd.memset(sw[:], 0.0)

        # ---- DMA program: issued first so the rings start streaming
        # immediately.  Each dma_start costs ~0.65us of serialized
        # descriptor generation on its ring (pipeline depth ~5), and
        # the two HWDGE rings round-robin the 16 SDMA engines at packet
        # granularity, so ring position IS priority: startup-critical
        # bytes go first on their ring and everything else queues
        # BEHIND them on the same rings.  X chunks ride in PAIRS (one
        # transfer per two k-chunks) to halve the descriptor-gen count
        # on the critical path.
        #   sync:   region-A pairs p0..p3, region-B pairs
        #   scalar: w0..w15 in process order, then Y drains
        KP = KC // 2
        xt = {}
        for r in range(nreg):
            for p in range(KP):
                xt[(r, p)] = xpool.tile(
                    [128, 2 * widths[r]], F16, tag=f"x{r}_{p}", name=f"x{r}_{p}"
                )
        for r in range(nreg):
            for p in range(KP):
                nc.sync.dma_start(xt[(r, p)][:], xs[(r, p)].ap())

        wt = {}
        for li in range(NL):
            t = wpool.tile([128, KC * 128], F16, tag=f"w{li}", name=f"w{li}")
            wt[li] = t
            nc.scalar.dma_start(t[:], wh.ap()[li])

        def wchunk(li, k):
            return wt[li][:, k * 128 : (k + 1) * 128]

        def xchunk(r, k, ws, wl):
            base = (k % 2) * widths[r]
            return xt[(r, k // 2)][:, base + ws : base + ws + wl]

        # ---- PE clock warm-up: any PE idle gap >~3.4us drops the clock
        # to 0.65GHz with a ~3.4us re-ramp, so keep the array busy on
        # scratch matmuls from kernel start until the first real window's
        # data has landed -- the ramp credit then carries into real work.
        spt = ppool.tile([128, WMAX], F32, tag="ps", name="spt")
        for _ in range(NWARM_128):
            nc.tensor.matmul(spt[:, :128], sw[:], sx[:, :128], start=True, stop=True)
        for _ in range(NWARM_256):
            nc.tensor.matmul(spt[:, :256], sw[:], sx[:], start=True, stop=True)

        # column offsets of each linear's Y block in the output tensor
        offs = []
        off = 0
        for li in range(NL):
            offs.append(off)
            off += F_ls[li]

        # ---- Region-major schedule: all linears' region-A windows run
        # before any region-B window.  Region A alone (plus the
        # streaming W planes) supplies tens of us of compute, so region
        # B lands long before it is needed.  The first linear's pass is
        # paced by the incoming chunk-pair stream; scratch matmuls
        # between its k-steps keep the PE's HAM activity window
        # accruing so the clock is warm when the pass ends.  Region B
        # runs largest-first so the final Y drain is the smallest
        # block.
        yts = {}
        ydrained = {li: 0 for li in range(NL)}
        winsl = {li: _windows(F_ls[li], bounds) for li in range(NL)}
        for r in range(nreg):
            order = [li for li in range(NL) if F_ls[li] > bounds[r]]
            if r == nreg - 1 and len(order) > 1:
                order = order[::-1]
            for li in order:
                wins = [w for w in winsl[li] if w[0] == r]
                if li not in yts:
                    yts[li] = ypool.tile(
                        [128, F_ls[li]], F16, tag=f"yt{li}", name=f"yt{li}"
                    )
                yt = yts[li]
                chase = r == 0 and li == order[0]
                for g0 in range(0, len(wins), 4):
                    grp = wins[g0 : g0 + 4]
                    pts = [
                        ppool.tile(
                            [128, WMAX], F32, tag="ps", name=f"ps{r}_{li}_{g0}_{j}"
                        )
                        for j in range(len(grp))
                    ]
                    for k in range(KC):
                        for (ri, ws, wl), pt in zip(grp, pts):
                            nc.tensor.matmul(
                                pt[:, :wl],
                                wchunk(li, k),
                                xchunk(r, k, ws, wl),
                                start=(k == 0),
                                stop=(k == KC - 1),
                            )
                        if chase and k % 2 == 1 and k < KC - 1:
                            # bridge to the next chunk-pair's arrival
                            for _ in range(5):
                                nc.tensor.matmul(
                                    spt[:, :128], sw[:], sx[:, :128],
                                    start=True, stop=True,
                                )
                    for (ri, ws, wl), pt in zip(grp, pts):
                        woff = bounds[ri] + ws
                        nc.vector.tensor_copy(yt[:, woff : woff + wl], pt[:, :wl])
                # drain this linear's completed span; finish on its last
                # region.  Mid-kernel drains ride the scalar ring; the
                # final block of the program goes on the (idle by then)
                # sync ring.
                done = min(bounds[r + 1], F_ls[li])
                last = done >= F_ls[li]
                dr = ydrained[li]
                if last or done - dr >= 688:
                    is_final = r == nreg - 1 and li == order[-1]
                    eng = nc.sync if is_final else nc.scalar
                    eng.dma_start(
                        y.ap()[:, offs[li] + dr : offs[li] + done],
                        yt[:, dr:done],
                    )
                    ydrained[li] = done

    nc.compile()
    return nc


# ---------------------------------------------------------------------------
# host wrapper
# ---------------------------------------------------------------------------

def _segment_structure(idx, T):
    t = np.arange(T)
    seg = np.searchsorted(idx, t, side="left")
    valid = seg < N
    segc = np.clip(seg, 0, N - 1)
    start = np.where(segc > 0, idx[np.maximum(segc - 1, 0)] + 1, 0)
    lidx = np.minimum(t - start, L - 1).astype(np.int64)
    lens = np.bincount(segc[valid], minlength=N)
    return t, seg, valid, segc, lidx, lens


def _install_ntff_hook():
    """Profiling-only: register the axon NTFF profile hook (dev use)."""
    import sys
    import types

    try:
        import antenv

        if "antenv.axon_hooks" not in sys.modules:
            mod = types.ModuleType("antenv.axon_hooks")
            holder = [None]
            mod.set_axon_ntff_profile_hook = lambda h: holder.__setitem__(0, h)
            mod.get_axon_ntff_profile_hook = lambda: holder[0]
            sys.modules["antenv.axon_hooks"] = mod
            antenv.axon_hooks = mod
            from trn_agent_boot.trn_boot import _ntff_profile_via_ctypes

            mod.set_axon_ntff_profile_hook(
                _ntff_profile_via_ctypes("/opt/axon/libaxon_pjrt.so")
            )
    except Exception as e:
        print(f"NTFF hook install failed: {e}")


def kernel(pooled_vectors, W, pooling_indices, target_length, _trace=False):
    pooled = np.asarray(pooled_vectors, dtype=np.float32)
    Wf = np.asarray(W, dtype=np.float32)
    idx = np.asarray(pooling_indices).astype(np.int64)
    T = int(np.asarray(target_length))

    t, seg, valid, segc, lidx, lens = _segment_structure(idx, T)

    order = np.argsort(-lens, kind="stable")      # segments by len desc
    rank_of_seg = np.empty(N, dtype=np.int64)
    rank_of_seg[order] = np.arange(N)
    N_l = (lens[None, :] > np.arange(L)[:, None]).sum(axis=1)  # (L,)

    # process order: ascending prefix length (early compute only needs
    # early X chunks); zero-size linears skipped
    proc = [l for l in np.argsort(N_l, kind="stable") if N_l[l] > 0]
    F_ls = [8 * int(N_l[l]) for l in proc]
    F = max(F_ls) if F_ls else 8

    nc = _build_program(F_ls)

    # flat column axis: (rank-major, batch-minor), ranks with len>0 only
    n0 = F // 8
    Xh = (
        pooled.transpose(2, 1, 0)[:, order[:n0], :]
        .reshape(D, F)
        .astype(np.float16)
    )  # (D, F), col = r*8 + b
    bounds = _bounds(F_ls)
    nreg = len(bounds) - 1
    # per-(region, chunk-pair) blocks, each (128, 2w): partition p holds
    # chunk 2p's row then chunk 2p+1's row, contiguous
    xblocks = {}
    for r in range(nreg):
        blk = Xh[:, bounds[r] : bounds[r + 1]]          # (1024, w)
        for p in range(KC // 2):
            xblocks[(r, p)] = np.ascontiguousarray(
                np.concatenate(
                    [
                        blk[(2 * p) * 128 : (2 * p + 1) * 128],
                        blk[(2 * p + 1) * 128 : (2 * p + 2) * 128],
                    ],
                    axis=1,
                )
            )

    in_maps = []
    for c in range(NCORES):
        # W[l, m_slice, d] -> (l, kp, kc*128+m) with d = kc*128 + kp
        wc = (
            Wf[np.array(proc), c * 128 : (c + 1) * 128, :]
            .transpose(0, 2, 1)
            .reshape(len(proc), KC, 128, 128)
            .transpose(0, 2, 1, 3)
            .reshape(len(proc), 128, KC * 128)
            .astype(np.float16)
        )
        im = {"w": np.ascontiguousarray(wc)}
        for (r, p), xb in xblocks.items():
            im[f"x{r}_{p}"] = xb
        in_maps.append(im)

    kwargs = {}
    if _trace:
        _install_ntff_hook()
        kwargs = dict(trace=True)
    res = run_bass_kernel_spmd(nc, in_maps, core_ids=list(range(NCORES)), **kwargs)
    results = res.results

    # assemble (D, total_cols) then scatter to (B, T, D)
    Yall = np.concatenate(
        [np.asarray(results[c]["y"]) for c in range(NCORES)], axis=0
    )  # (1024, total_cols) f16
    col_off = np.zeros(L, dtype=np.int64)
    off = 0
    for li, l in enumerate(proc):
        col_off[l] = off
        off += F_ls[li]

    Dout = Wf.shape[1]
    out = np.zeros((B, T, Dout), dtype=np.float32)
    tv = t[valid]
    ci = col_off[lidx[tv]] + rank_of_seg[segc[tv]] * 8  # (Tv,)
    cib = ci[:, None] + np.arange(B)[None, :]           # (Tv, B)
    out[:, tv, :] = Yall[:, cib].transpose(2, 1, 0).astype(np.float32)

    if _trace:
        kernel._last_exec_time_ns = res.exec_time_ns
        kernel._last_results = res
    return out
